# revision 22
# baseline (speedup 1.0000x reference)
"""Trainium2 Bass kernel for pre-LN multi-head attention block.

Reference computation (per batch element):
  xn = LayerNorm(x) * gamma + beta                 [N, D]
  qkv = xn @ w_qkv.T                               [N, 3*INNER]
  q, k, v -> [H, N, Dh]; attn = softmax(q k^T / sqrt(Dh)); o = attn @ v
  out = o @ w_proj.T + b_proj                      [N, D]

Sharding: data-parallel over batch B=8 across the 8 NeuronCores (one batch
element per core, no collectives).

Shapes (hardcoded): B=8, N=2048, D=512, H=8, Dh=64, INNER=512.
"""

import os
import numpy as np
import ml_dtypes

import concourse.bass as bass
import concourse.mybir as mybir
import concourse.tile as tile
from concourse import bacc, masks

F32 = mybir.dt.float32
BF16 = mybir.dt.bfloat16

B = 8
N = 2048
D = 512
H = 8
Dh = 64
INNER = H * Dh  # 512
EPS = 1e-6
SCALE = Dh ** -0.5  # 0.125

P = 128
NT = N // P       # 16 token tiles
DC = D // P       # 4 d-chunks
QT = 4            # q tiles of 512
QW = N // QT      # 512
KC = N // P       # 16 key chunks of 128
HT = H // 2       # 4 head pairs (2 heads share a 128-partition tile)


def build_graph():
    nc = bacc.Bacc()

    x = nc.declare_dram_parameter("x", [N, D], F32, isOutput=False)
    w_qkvT = nc.declare_dram_parameter("w_qkvT", [D, 3 * INNER], BF16, isOutput=False)
    b_qkv = nc.declare_dram_parameter("b_qkv", [3 * INNER], F32, isOutput=False)
    w_projT = nc.declare_dram_parameter("w_projT", [INNER, D], BF16, isOutput=False)
    b_proj = nc.declare_dram_parameter("b_proj", [D], F32, isOutput=False)
    out = nc.declare_dram_parameter("out", [N, D], F32, isOutput=True)

    xn_dram = nc.dram_tensor("xn_scratch", [N, D], BF16)

    def bcast_ap(ap_1d, parts):
        # DRAM [D] -> [parts, D] partition-broadcast access pattern
        return bass.AP(tensor=ap_1d.tensor, offset=ap_1d.offset,
                       ap=[[0, parts]] + list(ap_1d.ap))

    with tile.TileContext(nc) as tc:
        with (
            tc.tile_pool(name="consts", bufs=1) as consts,
            tc.tile_pool(name="big", bufs=1) as big,
            tc.tile_pool(name="ln", bufs=3) as ln,
            tc.tile_pool(name="xload", bufs=4) as xload,
            tc.tile_pool(name="yout", bufs=4) as yout,
            tc.tile_pool(name="work", bufs=3) as work,
            tc.tile_pool(name="small", bufs=4) as small,
            tc.tile_pool(name="s_ps", bufs=2, space="PSUM") as s_ps,
            tc.tile_pool(name="o_ps", bufs=2, space="PSUM") as o_ps,
        ):
            # o_ps holds two [128, 512] f32 tags (po_a / po_b); the QKV /
            # proj / transpose phases borrow its slots (same bank budget).
            _mm_ctr = [0]

            def mm_ps_tile(shape, dtype):
                _mm_ctr[0] += 1
                tag = "po_a" if _mm_ctr[0] % 2 else "po_b"
                return o_ps.tile(shape, dtype, tag=tag,
                                 name=f"mm_{_mm_ctr[0]}")

            # ---- constants (scalar-engine DMA queue; x loads use sync) ----
            wq = consts.tile([P, DC, 3 * INNER], BF16)
            nc.scalar.dma_start(wq, w_qkvT.rearrange("(o p) f -> p o f", p=P))
            wp = consts.tile([P, DC, D], BF16)
            nc.scalar.dma_start(wp, w_projT.rearrange("(o p) f -> p o f", p=P))
            # qkv bias: per-feature column layout [128, 12] (feature tiles)
            bqkv_col = consts.tile([P, 3 * INNER // P], F32)
            nc.scalar.dma_start(bqkv_col, b_qkv.rearrange("(o p) -> p o", p=P))
            # v bias along free axis, broadcast across partitions
            bv_bc = consts.tile([P, INNER], F32)
            nc.scalar.dma_start(bv_bc, bcast_ap(b_qkv[2 * INNER:3 * INNER], P))
            bias_bc = consts.tile([P, D], F32)
            nc.scalar.dma_start(bias_bc, bcast_ap(b_proj[:], P))
            eps_t = consts.tile([P, 1], F32)
            nc.vector.memset(eps_t, EPS)

            # ---- LayerNorm + transpose: x -> xnT_s [128, DC, QW] bf16 x4 ----
            # (gamma/beta are folded into w_qkv / b_qkv on the host)
            xnT = [big.tile([P, DC, QW], BF16, name=f"xnT{s}") for s in range(QT)]
            for i in range(NT):
                xt = xload.tile([P, D], F32)
                nc.sync.dma_start(xt, x[i * P:(i + 1) * P, :])
                stats = ln.tile([P, 6], F32)
                nc.vector.bn_stats(stats, xt)
                mv = ln.tile([P, 2], F32)
                nc.vector.bn_aggr(mv, stats)
                std = ln.tile([P, 1], F32)
                nc.scalar.activation(std, mv[:, 1:2],
                                     mybir.ActivationFunctionType.Sqrt,
                                     bias=eps_t)
                rstd = ln.tile([P, 1], F32)
                nc.vector.reciprocal(rstd, std)
                xn_b = ln.tile([P, D], BF16)
                nc.vector.tensor_scalar(out=xn_b, in0=xt,
                                        scalar1=mv[:, 0:1], scalar2=rstd,
                                        op0=mybir.AluOpType.subtract,
                                        op1=mybir.AluOpType.mult)
                nc.scalar.dma_start(xn_dram[i * P:(i + 1) * P, :], xn_b)
                if i % 4 == 3:
                    s = i // 4
                    nc.sync.dma_start_transpose(
                        xnT[s][:, :, :], xn_dram[s * QW:(s + 1) * QW, :])

            # ---- QKV projections (s-chunk pipelined behind LN) ----
            qT = big.tile([P, HT, N], BF16)
            kT = big.tile([P, HT, N], BF16)
            v_aug = big.tile([P, KC, H, Dh + 1], BF16)
            nc.vector.memset(v_aug[:, :, :, Dh:Dh + 1], 1.0)

            for s in range(QT):
                for f in range(2 * HT):  # Q: f 0-3, K: f 4-7
                    dest = qT if f < HT else kT
                    ft = f % HT
                    ps = mm_ps_tile([P, QW], F32)
                    for dc in range(DC):
                        nc.tensor.matmul(ps,
                                         lhsT=wq[:, dc, f * P:(f + 1) * P],
                                         rhs=xnT[s][:, dc, :],
                                         start=(dc == 0), stop=(dc == DC - 1))
                    # copy + per-feature-row qkv bias (DVE, fused cast)
                    nc.vector.tensor_scalar(
                        out=dest[:, ft, s * QW:(s + 1) * QW], in0=ps,
                        scalar1=bqkv_col[:, f:f + 1], scalar2=None,
                        op0=mybir.AluOpType.add)
                for j in range(4):  # V branch for this s-chunk
                    nt = s * 4 + j
                    ps = mm_ps_tile([P, INNER], F32)
                    for dc in range(DC):
                        nc.tensor.matmul(ps,
                                         lhsT=xnT[s][:, dc, j * P:(j + 1) * P],
                                         rhs=wq[:, dc, 2 * INNER:3 * INNER],
                                         start=(dc == 0), stop=(dc == DC - 1))
                    nc.vector.tensor_tensor(
                        v_aug[:, nt, :, 0:Dh],
                        ps[:, :].rearrange("p (h c) -> p h c", h=H),
                        bv_bc[:, :].rearrange("p (h c) -> p h c", h=H),
                        mybir.AluOpType.add)

            # ---- attention (s outer so proj can drain per s-chunk) ----
            oT = [big.tile([P, DC, QW], BF16, name=f"oT{s}") for s in range(QT)]

            for t in range(HT):
                for s in range(QT):
                    po_a = o_ps.tile([P, QW], F32, tag="po_a")
                    po_b = o_ps.tile([P, QW], F32, tag="po_b")
                    for kc in range(KC):
                        ps = s_ps.tile([P, 2 * QW], F32)
                        nc.tensor.matmul(
                            ps[:, 0:QW],
                            lhsT=kT[0:Dh, t, kc * P:(kc + 1) * P],
                            rhs=qT[0:Dh, t, s * QW:(s + 1) * QW],
                            start=True, stop=True)
                        nc.tensor.matmul(
                            ps[:, QW:2 * QW],
                            lhsT=kT[Dh:P, t, kc * P:(kc + 1) * P],
                            rhs=qT[Dh:P, t, s * QW:(s + 1) * QW],
                            start=True, stop=True)
                        et = work.tile([P, 2 * QW], BF16)
                        nc.scalar.activation(et, ps,
                                             mybir.ActivationFunctionType.Exp,
                                             scale=SCALE)
                        nc.tensor.matmul(po_a[0:Dh + 1, :],
                                         lhsT=v_aug[:, kc, 2 * t, :],
                                         rhs=et[:, 0:QW],
                                         start=(kc == 0), stop=(kc == KC - 1))
                        nc.tensor.matmul(po_b[0:Dh + 1, :],
                                         lhsT=v_aug[:, kc, 2 * t + 1, :],
                                         rhs=et[:, QW:2 * QW],
                                         start=(kc == 0), stop=(kc == KC - 1))
                    # normalize: O = O~ / rowsum (rowsum in row 64).
                    for h_off, po in ((0, po_a), (1, po_b)):
                        rs = small.tile([1, QW], F32, tag="rs")
                        nc.vector.tensor_copy(rs, po[Dh:Dh + 1, :])
                        ot_tmp = small.tile([Dh, QW], BF16, tag="ot_tmp")
                        nc.vector.tensor_copy(ot_tmp, po[0:Dh, :])
                        rr = small.tile([1, QW], F32, tag="rr")
                        nc.vector.reciprocal_approx_fast(out=rr, in_=rs)
                        rb = small.tile([Dh, QW], F32, tag="rb")
                        nc.gpsimd.partition_broadcast(rb, rr)
                        nc.vector.tensor_tensor(
                            oT[s][h_off * Dh:(h_off + 1) * Dh, t, :],
                            ot_tmp, rb, mybir.AluOpType.mult)


            for nt in range(NT):
                ps = mm_ps_tile([P, D], F32)
                for c in range(DC):
                    nc.tensor.matmul(ps,
                                     lhsT=oT[nt // 4][:, c, (nt % 4) * P:(nt % 4 + 1) * P],
                                     rhs=wp[:, c, :],
                                     start=(c == 0), stop=(c == DC - 1))
                yt = yout.tile([P, D], F32, tag="yt", name=f"yt_{nt}")
                nc.vector.tensor_tensor(yt, ps, bias_bc, mybir.AluOpType.add)
                nc.sync.dma_start(out[nt * P:(nt + 1) * P, :], yt)

    nc.compile()
    return nc


_CACHED = {}


def _prep_weights(w_qkv, w_proj, b_proj, ln_gamma, ln_beta):
    # Fold LN affine into the QKV projection:
    #   (xn * gamma + beta) @ W^T == xn @ (W * gamma)^T + beta @ W^T
    w_qkv = np.asarray(w_qkv, dtype=np.float64)
    gamma = np.asarray(ln_gamma, dtype=np.float64)
    beta = np.asarray(ln_beta, dtype=np.float64)
    w_eff = w_qkv * gamma[None, :]
    b_qkv = w_qkv @ beta
    return {
        "w_qkvT": np.ascontiguousarray(w_eff.T).astype(ml_dtypes.bfloat16),
        "b_qkv": np.ascontiguousarray(b_qkv).astype(np.float32),
        "w_projT": np.ascontiguousarray(np.asarray(w_proj).T).astype(ml_dtypes.bfloat16),
        "b_proj": np.ascontiguousarray(b_proj).astype(np.float32),
    }


def kernel(x, w_qkv, w_proj, b_proj, ln_gamma, ln_beta):
    from concourse.bass_utils import run_bass_kernel_spmd

    x = np.asarray(x, dtype=np.float32)
    assert x.shape == (B, N, D), x.shape

    if "nc" not in _CACHED:
        _CACHED["nc"] = build_graph()
    nc = _CACHED["nc"]

    shared = _prep_weights(np.asarray(w_qkv), np.asarray(w_proj),
                           np.asarray(b_proj), np.asarray(ln_gamma),
                           np.asarray(ln_beta))
    in_maps = [dict(shared, x=np.ascontiguousarray(x[i])) for i in range(B)]

    trace = bool(int(os.environ.get("KERNEL_TRACE", "0")))
    res = run_bass_kernel_spmd(nc, in_maps, core_ids=list(range(B)),
                               trace=trace)
    if trace:
        _CACHED["exec_time_ns"] = res.exec_time_ns
        _CACHED["last_result"] = res
    outs = [np.asarray(res.results[i]["out"], dtype=np.float32)
            for i in range(B)]
    return np.stack(outs, axis=0)
